# revision 36
# baseline (speedup 1.0000x reference)
import sys
import numpy as np

sys.path.insert(0, "/opt/trn_rl_repo")
sys.path.insert(0, "/opt/trn_rl_repo/concourse")

import ml_dtypes
import concourse.bass as bass
import concourse.bacc as bacc
import concourse.mybir as mybir
import concourse.tile as tile
from concourse.bass import IndirectOffsetOnAxis
from concourse.bass_utils import run_bass_kernel_spmd
from concourse.masks import make_identity

F32 = mybir.dt.float32
BF16 = mybir.dt.bfloat16
I32 = mybir.dt.int32
I16 = mybir.dt.int16
NPBF = ml_dtypes.bfloat16

N = 20000
E = 160000
B = 128
NDEV = 8
NPD = N // NDEV          # 2500 nodes per device
NT = (NPD + 127) // 128  # 20 dst tiles per device
H = 4
C1IN, C1 = 768, 512
C2IN, C2 = 512, 256
XROW = 784              # [x(768) | as1(4) ad1(4) | pad8] bf16 -> 1568B rows
T2W = 528                # [h1(512) | as2(4) ad2(4) | pad8] bf16 -> 1056B rows
NEG = 0.2
TS1, TS2 = 8, 16         # L1 tile boundaries for the 3-way t2 AllGather
R1 = TS1 * 128           # 896 local rows in part A
R2 = TS2 * 128           # parts: A=[0,896) B=[896,1792) C=[1792,2500)


def _host_prep(edge_index, batch):
    """Integer-only preprocessing: balanced node partition, edge chunk layout."""
    import heapq
    src = np.concatenate([edge_index[0], np.arange(N, dtype=np.int64)]).astype(np.int64)
    dst = np.concatenate([edge_index[1], np.arange(N, dtype=np.int64)]).astype(np.int64)

    # -- degree-balanced assignment of nodes to the 8*NT (device, tile) slots --
    deg = np.bincount(dst, minlength=N)
    tile_rows = [128] * (NT - 1) + [NPD - 128 * (NT - 1)]
    rem = np.array([[r for r in tile_rows] for _ in range(NDEV)], dtype=np.int64)
    fill = np.zeros((NDEV, NT), dtype=np.int64)
    perm_d = np.empty(N, np.int64)
    perm_t = np.empty(N, np.int64)
    perm_s = np.empty(N, np.int64)
    heap = [(0, d * NT + t) for d in range(NDEV) for t in range(NT)]
    heapq.heapify(heap)
    for n in np.argsort(-deg, kind="stable"):
        while True:
            s, dt = heapq.heappop(heap)
            d, t = divmod(dt, NT)
            if rem[d, t] > 0:
                break
        perm_d[n], perm_t[n], perm_s[n] = d, t, fill[d, t]
        fill[d, t] += 1
        rem[d, t] -= 1
        heapq.heappush(heap, (s + int(deg[n]), dt))
    perm_l = perm_t * 128 + perm_s

    ed, et, es = perm_d[dst], perm_t[dst], perm_s[dst]
    cnt = np.zeros((NDEV, NT), dtype=np.int64)
    np.add.at(cnt, (ed, et), 1)
    Ks = [max(1, int(np.ceil(cnt[:, t].max() / 128.0))) for t in range(NT)]
    SK = sum(Ks)
    offs = np.cumsum([0] + Ks)

    # L2 gather table: [devs' rows < R1 | devs' rows in [R1,R2) | devs' rest]
    sd, sl = perm_d[src], perm_l[src]
    src2 = np.where(
        sl < R1, sd * R1 + sl,
        np.where(sl < R2, NDEV * R1 + sd * (R2 - R1) + (sl - R1),
                 NDEV * R2 + sd * (NPD - R2) + (sl - R2)))

    # flat src index per (device, tile): edge j -> lane j%128, chunk j//128
    xidx32 = np.zeros((NDEV, 128, SK), dtype=np.int32)   # L1
    xidx32b = np.zeros((NDEV, 128, SK), dtype=np.int32)  # L2
    dstf = np.full((NDEV, 128, SK), -1.0, dtype=NPBF)
    dstfR = np.full((NDEV, SK, 128), -1.0, dtype=NPBF)   # chunk-major rows

    for d in range(NDEV):
        m = ed == d
        s_d, s2_d, t_d, dl_d = src[m], src2[m], et[m], es[m]
        for t in range(NT):
            mt = t_d == t
            s_t, s2_t = s_d[mt], s2_d[mt]
            dl_t = dl_d[mt]
            o = offs[t]
            j = np.arange(len(s_t))
            xidx32[d, j % 128, o + j // 128] = s_t
            xidx32b[d, j % 128, o + j // 128] = s2_t
            dstf[d, j % 128, o + j // 128] = dl_t.astype(NPBF)
            dstfR[d, o + j // 128, j % 128] = dl_t.astype(NPBF)

    batchf = np.full((NDEV, 128, NT), -1.0, dtype=NPBF)
    b_np = np.asarray(batch).astype(np.int64)
    batchf[perm_d, perm_s, perm_t] = b_np.astype(NPBF)

    return Ks, offs, SK, xidx32, xidx32b, dstf, dstfR, batchf, perm_d, perm_l


def _build_A(a_src, a_dst, cph):
    A = np.zeros((H * cph, 8), dtype=np.float32)
    for h in range(H):
        A[h * cph:(h + 1) * cph, h] = a_src[h]
        A[h * cph:(h + 1) * cph, 4 + h] = a_dst[h]
    return A


def _build(Ks, offs, SK):
    """Emit the Bass program (identical for all 8 cores)."""
    nc = bacc.Bacc("TRN2", target_bir_lowering=False, debug=False, num_devices=NDEV)

    # ---- I/O ----
    xa_t = nc.dram_tensor("xa", [N, XROW], BF16, kind="ExternalInput")
    W1_t = nc.dram_tensor("W1", [C1IN, H * C1], BF16, kind="ExternalInput")
    W2_t = nc.dram_tensor("W2", [C2IN, H * C2], BF16, kind="ExternalInput")
    wa2_t = nc.dram_tensor("wa2", [C2IN, 8], BF16, kind="ExternalInput")
    fcW_t = nc.dram_tensor("fcW", [C2, 2], F32, kind="ExternalInput")
    b1_t = nc.dram_tensor("b1", [C1], BF16, kind="ExternalInput")
    b2_t = nc.dram_tensor("b2", [C2], BF16, kind="ExternalInput")
    fcb_t = nc.dram_tensor("fcb", [2], F32, kind="ExternalInput")
    xidx32_t = nc.dram_tensor("xidx32", [128, SK], I32, kind="ExternalInput")
    xidx32b_t = nc.dram_tensor("xidx32b", [128, SK], I32, kind="ExternalInput")
    dstf_t = nc.dram_tensor("dstf", [128, SK], BF16, kind="ExternalInput")
    dstfR_t = nc.dram_tensor("dstfR", [SK, 128], BF16, kind="ExternalInput")
    batchf_t = nc.dram_tensor("batchf", [128, NT], BF16, kind="ExternalInput")
    asadloc_t = nc.dram_tensor("asadloc", [NPD, 8], BF16, kind="ExternalInput")
    y_t = nc.dram_tensor("y", [B, 2], F32, kind="ExternalOutput")

    # ---- internal DRAM ----
    t2_locA = nc.dram_tensor("t2_locA", [R1, T2W], BF16)
    t2_locB = nc.dram_tensor("t2_locB", [R2 - R1, T2W], BF16)
    t2_locC = nc.dram_tensor("t2_locC", [NPD - R2, T2W], BF16)
    t2_full = nc.dram_tensor("t2_full", [N, T2W], BF16, addr_space="Shared")
    pc_loc = nc.dram_tensor("pc_loc", [B, C2 + 1], F32)
    pc_red = nc.dram_tensor("pc_red", [B, C2 + 1], F32, addr_space="Shared")

    RG = [list(range(NDEV))]
    KMAX = max(Ks)

    with tile.TileContext(nc) as tc:
        with (
            tc.tile_pool(name="const", bufs=1) as cp,
            tc.tile_pool(name="small", bufs=3) as sp,
            tc.tile_pool(name="selp", bufs=3) as selp,
            tc.tile_pool(name="selTp", bufs=3) as selTp,
            tc.tile_pool(name="dstfTp", bufs=3) as dstfTp,
            tc.tile_pool(name="selwp", bufs=3) as selwp,
            tc.tile_pool(name="xgp", bufs=4) as xgp,
            tc.tile_pool(name="utp", bufs=3) as utp,
            tc.tile_pool(name="psu", bufs=2, space="PSUM") as psu,
            tc.tile_pool(name="psh", bufs=1, space="PSUM") as psh,
            tc.tile_pool(name="psa", bufs=2, space="PSUM") as psa,
        ):
            # ================= constants =================
            ident = cp.tile([128, 128], F32, tag="ident")
            make_identity(nc, ident[:])
            iota_i = cp.tile([128, 128], I32, tag="iota_i")
            nc.gpsimd.iota(iota_i[:], pattern=[[1, 128]], base=0, channel_multiplier=0)
            iotaTb = cp.tile([128, 128], BF16, tag="iotaTb")
            nc.vector.tensor_copy(out=iotaTb[:], in_=iota_i[:])
            iota_ci = cp.tile([128, 1], I32, tag="iota_ci")
            nc.gpsimd.iota(iota_ci[:], pattern=[[1, 1]], base=0, channel_multiplier=1)
            iotaCb = cp.tile([128, 1], BF16, tag="iotaCb")
            nc.vector.tensor_copy(out=iotaCb[:], in_=iota_ci[:])
            ones1 = cp.tile([1, 128], BF16, tag="ones1")
            nc.vector.memset(ones1[:], 1.0)
            ones1f = cp.tile([1, 128], F32, tag="ones1f")
            nc.vector.memset(ones1f[:], 1.0)

            b1_sb = cp.tile([1, C1], BF16, tag="b1")
            nc.sync.dma_start(out=b1_sb[:], in_=b1_t[None, :])
            b2_sb = cp.tile([1, C2], BF16, tag="b2")
            nc.sync.dma_start(out=b2_sb[:], in_=b2_t[None, :])
            fcb_sb = cp.tile([1, 2], F32, tag="fcb")
            nc.sync.dma_start(out=fcb_sb[:], in_=fcb_t[None, :])
            fcW_sb = cp.tile([128, 4], F32, tag="fcW")
            for c in range(2):
                nc.sync.dma_start(out=fcW_sb[:, 2 * c:2 * c + 2],
                                  in_=fcW_t[c * 128:(c + 1) * 128, :])
            wa2blk = cp.tile([128, 4 * 8], BF16, tag="wa2blk")
            nc.sync.dma_start(
                out=wa2blk[:].rearrange("p (c j) -> p c j", j=8),
                in_=wa2_t[:].rearrange("(c p) j -> p c j", p=128))

            CC1 = C1IN // 128  # 6
            CC2 = C2IN // 128  # 4
            W1_sb = cp.tile([128, CC1 * H * C1], BF16, tag="W1")
            for c in range(CC1):
                nc.sync.dma_start(out=W1_sb[:, c * H * C1:(c + 1) * H * C1],
                                  in_=W1_t[c * 128:(c + 1) * 128, :])
            W2_sb = cp.tile([128, CC2 * H * C2], BF16, tag="W2")
            for c in range(CC2):
                nc.sync.dma_start(out=W2_sb[:, c * H * C2:(c + 1) * H * C2],
                                  in_=W2_t[c * 128:(c + 1) * 128, :])

            # ================= helper: one GAT layer sweep =================
            # Software-pipelined: stage_a (loads/gather/sel) runs 2 tiles
            # ahead, stage_b1/b2 (attention) 1 tile ahead of stage_c (agg).
            def gat_sweep(layer):
                if layer == 1:
                    CIN, CC, COUT = C1IN, CC1, C1
                    ROW = XROW
                    W_sb, b_sb = W1_sb, b1_sb
                    x_tab, idx_t = xa_t, xidx32_t
                else:
                    CIN, CC, COUT = C2IN, CC2, C2
                    ROW = T2W
                    W_sb, b_sb = W2_sb, b2_sb
                    x_tab, idx_t = t2_full, xidx32b_t

                poolacc = None
                if layer == 2:
                    poolacc = cp.tile([128, C2 + 1], F32, tag="poolacc")
                    nc.vector.memset(poolacc[:], 0.0)

                st = {}

                def t2part(t):
                    if t < TS1:
                        return t2_locA, t * 128
                    if t < TS2:
                        return t2_locB, t * 128 - R1
                    return t2_locC, t * 128 - R2

                def stage_a(t):
                    K, o = Ks[t], offs[t]
                    rows = min(128, NPD - t * 128)
                    xg = xgp.tile([128, KMAX * XROW], BF16, tag="xg")
                    idx_sb = sp.tile([128, KMAX], I32, tag="idx32")
                    nc.sync.dma_start(out=idx_sb[:, :K],
                                      in_=idx_t[:, o:o + K])
                    for k in range(K):
                        nc.gpsimd.indirect_dma_start(
                            out=xg[:, k * ROW:(k + 1) * ROW], out_offset=None,
                            in_=x_tab[:, :],
                            in_offset=IndirectOffsetOnAxis(
                                ap=idx_sb[:, k:k + 1], axis=0))
                    dstf_sb = sp.tile([128, KMAX], BF16, tag="dstf")
                    nc.sync.dma_start(out=dstf_sb[:, :K], in_=dstf_t[:, o:o + K])
                    ad_sb = sp.tile([128, 4], BF16, tag="ad")
                    if rows < 128:
                        nc.vector.memset(ad_sb[:], 0.0)
                    if layer == 1:
                        nc.sync.dma_start(out=ad_sb[:rows, :],
                                          in_=asadloc_t[t * 128:t * 128 + rows, 4:8])
                    else:
                        t2d, r0 = t2part(t)
                        nc.sync.dma_start(out=ad_sb[:rows, :],
                                          in_=t2d[r0:r0 + rows,
                                                  C2IN + 4:C2IN + 8])
                    dstfT = dstfTp.tile([128, KMAX * 128], BF16, tag="dstfT")
                    nc.sync.dma_start(
                        out=dstfT[:, :K * 128].rearrange("p (k d) -> p k d", d=128),
                        in_=dstfR_t[None, o:o + K, :].to_broadcast([128, K, 128]))
                    sel = selp.tile([128, KMAX * 128], BF16, tag="sel")
                    nc.vector.tensor_tensor(
                        out=sel[:, :K * 128].rearrange("p (k d) -> p k d", d=128),
                        in0=dstf_sb[:, :K][:, :, None].to_broadcast([128, K, 128]),
                        in1=iotaTb[:, None, :].to_broadcast([128, K, 128]),
                        op=mybir.AluOpType.is_equal)
                    selT = selTp.tile([128, KMAX * 128], BF16, tag="selT")
                    nc.vector.tensor_tensor(
                        out=selT[:, :K * 128],
                        in0=iotaCb[:, :1].to_broadcast([128, K * 128]),
                        in1=dstfT[:, :K * 128], op=mybir.AluOpType.is_equal)
                    st[t] = dict(xg=xg, sel=sel, selT=selT, ad=ad_sb)

                def stage_b1(t):
                    K = Ks[t]
                    s = st[t]
                    xg, sel, selT, ad_sb = s["xg"], s["sel"], s["selT"], s["ad"]
                    ade_ps = psa.tile([128, 257], F32, tag="attn")
                    for k in range(K):
                        nc.tensor.matmul(out=ade_ps[:, 4 * k:4 * k + 4],
                                         lhsT=selT[:, 128 * k:128 * (k + 1)],
                                         rhs=ad_sb[:],
                                         start=(k == 0), stop=(k == K - 1))
                    xg3 = xg[:, :K * ROW].rearrange("p (k d) -> p k d", d=ROW)
                    z = sp.tile([128, 4 * KMAX], F32, tag="z")
                    z3 = z[:, :4 * K].rearrange("p (k s) -> p k s", s=4)
                    nc.vector.tensor_tensor(out=z3, in0=xg3[:, :, CIN:CIN + 4],
                                            in1=ade_ps[:, :4 * K].rearrange(
                                                "p (k s) -> p k s", s=4),
                                            op=mybir.AluOpType.add)
                    zs = sp.tile([128, 4 * KMAX], F32, tag="zs")
                    nc.vector.tensor_scalar_mul(zs[:, :4 * K], z[:, :4 * K], NEG)
                    nc.vector.tensor_tensor(out=z[:, :4 * K], in0=z[:, :4 * K],
                                            in1=zs[:, :4 * K], op=mybir.AluOpType.max)
                    ex = sp.tile([128, 4 * KMAX], BF16, tag="ex")
                    nc.scalar.activation(out=ex[:, :4 * K], in_=z[:, :4 * K],
                                         func=mybir.ActivationFunctionType.Exp)
                    s["ex"] = ex

                def stage_b2(t):
                    K = Ks[t]
                    s = st[t]
                    sel, selT, ex = s["sel"], s["selT"], s["ex"]
                    den_ps = psa.tile([128, 257], F32, tag="attn")
                    for k in range(K):
                        nc.tensor.matmul(out=den_ps[:, 0:4],
                                         lhsT=sel[:, 128 * k:128 * (k + 1)],
                                         rhs=ex[:, 4 * k:4 * k + 4],
                                         start=(k == 0), stop=(k == K - 1))
                    den_sb = sp.tile([128, 4], BF16, tag="den")
                    nc.scalar.activation(out=den_sb[:], in_=den_ps[:, 0:4],
                                         func=mybir.ActivationFunctionType.Copy)
                    dene_ps = psh.tile([128, 1024], F32, tag="hold")
                    for k in range(K):
                        nc.tensor.matmul(out=dene_ps[:, 4 * k:4 * k + 4],
                                         lhsT=selT[:, 128 * k:128 * (k + 1)],
                                         rhs=den_sb[:],
                                         start=(k == 0), stop=(k == K - 1))
                    rden = sp.tile([128, 4 * KMAX], F32, tag="rden")
                    nc.vector.tensor_scalar(out=rden[:, :4 * K],
                                            in0=dene_ps[:, :4 * K],
                                            scalar1=4.0, scalar2=1e-20,
                                            op0=mybir.AluOpType.mult,
                                            op1=mybir.AluOpType.max)
                    nc.vector.reciprocal(out=rden[:, :4 * K], in_=rden[:, :4 * K])
                    alpha = sp.tile([128, 4 * KMAX], BF16, tag="alpha")
                    nc.vector.tensor_tensor(out=alpha[:, :4 * K], in0=ex[:, :4 * K],
                                            in1=rden[:, :4 * K],
                                            op=mybir.AluOpType.mult)
                    selw = selwp.tile([128, KMAX * 512], BF16, tag="selw")
                    nc.vector.tensor_tensor(
                        out=selw[:, :K * 512].rearrange(
                            "p (k h d) -> p k h d", h=4, d=128),
                        in0=sel[:, :K * 128].rearrange(
                            "p (k d) -> p k d", d=128)[:, :, None, :].to_broadcast(
                            [128, K, 4, 128]),
                        in1=alpha[:, :4 * K].rearrange(
                            "p (k h) -> p k h", h=4)[:, :, :, None].to_broadcast(
                            [128, K, 4, 128]),
                        op=mybir.AluOpType.mult)
                    s["selw"] = selw

                def stage_c(t):
                    K = Ks[t]
                    rows = min(128, NPD - t * 128)
                    s = st.pop(t)
                    xg, selw = s["xg"], s["selw"]
                    out_ps = psh.tile([128, 1024], F32, tag="hold")
                    firsts = [True, True]
                    NPASS = CC // 2
                    for p_i in range(NPASS):
                        ut_ps = psu.tile([128, 1024], F32, tag="ut")
                        for k in range(K):
                            for ci in range(2):
                                c = 2 * p_i + ci
                                nc.tensor.matmul(
                                    out=ut_ps[:, ci * 512:(ci + 1) * 512],
                                    lhsT=xg[:, k * ROW + c * 128:
                                            k * ROW + (c + 1) * 128],
                                    rhs=selw[:, k * 512:(k + 1) * 512],
                                    start=(k == 0), stop=(k == K - 1))
                        ut_sb = utp.tile([128, 1024], BF16, tag="ut")
                        nc.scalar.activation(out=ut_sb[:],
                                             in_=ut_ps[:],
                                             func=mybir.ActivationFunctionType.Copy)
                        for ci in range(2):
                            c = 2 * p_i + ci
                            for h in range(H):
                                half = h % 2
                                last1 = (half == 1 and p_i == NPASS - 1
                                         and ci == 1 and h == 3)
                                nc.tensor.matmul(
                                    out=out_ps[:, half * 512:half * 512 + COUT],
                                    lhsT=ut_sb[:, ci * 512 + h * 128:
                                               ci * 512 + (h + 1) * 128],
                                    rhs=W_sb[:, c * H * COUT + h * COUT:
                                             c * H * COUT + (h + 1) * COUT],
                                    start=firsts[half], stop=last1)
                                firsts[half] = False
                    nc.tensor.matmul(out=out_ps[:, 0:COUT], lhsT=ones1[:],
                                     rhs=b_sb[:], start=False, stop=True)

                    oh1 = sp.tile([128, COUT], F32, tag="oh1")
                    nc.scalar.activation(out=oh1[:], in_=out_ps[:, 512:512 + COUT],
                                         func=mybir.ActivationFunctionType.Copy)
                    if layer == 1:
                        h1f_sb = sp.tile([128, C1], F32, tag="h1f")
                        nc.vector.tensor_tensor(out=h1f_sb[:],
                                                in0=out_ps[:, 0:COUT],
                                                in1=oh1[:],
                                                op=mybir.AluOpType.add)
                        h1_sb = sp.tile([128, C1], BF16, tag="h1")
                        nc.scalar.activation(out=h1_sb[:], in_=h1f_sb[:],
                                             func=mybir.ActivationFunctionType.Copy)
                        # asad2 = h1 @ wa2 via PE transposes of h1; the
                        # transpose scratch lives in the second bank of the
                        # same hold tile so psa stays a clean ade/den rotation
                        as2_ps = psh.tile([128, 1024], F32, tag="hold")
                        for c in range(CC2):
                            nc.tensor.transpose(out=as2_ps[:, 512:640],
                                                in_=h1f_sb[:, c * 128:(c + 1) * 128],
                                                identity=ident[:])
                            h1T = sp.tile([128, 128], BF16, tag="h1T")
                            nc.vector.tensor_copy(out=h1T[:], in_=as2_ps[:, 512:640])
                            nc.tensor.matmul(out=as2_ps[:, 0:8], lhsT=h1T[:],
                                             rhs=wa2blk[:, c * 8:(c + 1) * 8],
                                             start=(c == 0), stop=(c == CC2 - 1))
                        as2_sb = sp.tile([128, 8], BF16, tag="as2")
                        nc.vector.tensor_copy(out=as2_sb[:], in_=as2_ps[:, 0:8])
                        t2d, r0 = t2part(t)
                        nc.sync.dma_start(out=t2d[r0:r0 + rows, 0:C2IN],
                                          in_=h1_sb[:rows, :])
                        nc.sync.dma_start(out=t2d[r0:r0 + rows, C2IN:C2IN + 8],
                                          in_=as2_sb[:rows, :])
                        if t == TS1 - 1:
                            nc.gpsimd.collective_compute(
                                "AllGather", mybir.AluOpType.bypass,
                                replica_groups=RG,
                                ins=[t2_locA[:, :]],
                                outs=[t2_full[0:NDEV * R1, :]])
                        elif t == TS2 - 1:
                            nc.gpsimd.collective_compute(
                                "AllGather", mybir.AluOpType.bypass,
                                replica_groups=RG,
                                ins=[t2_locB[:, :]],
                                outs=[t2_full[NDEV * R1:NDEV * R2, :]])
                    else:
                        h2_sb = sp.tile([128, C2 + 1], BF16, tag="h2")
                        nc.vector.tensor_tensor(out=h2_sb[:, :C2],
                                                in0=out_ps[:, 0:COUT],
                                                in1=oh1[:, :COUT],
                                                op=mybir.AluOpType.add)
                        nc.vector.memset(h2_sb[:, C2:C2 + 1], 1.0)
                        selB = sp.tile([128, 128], BF16, tag="selB")
                        nc.vector.tensor_tensor(
                            out=selB[:],
                            in0=batchf_sb[:, t:t + 1].to_broadcast([128, 128]),
                            in1=iotaTb[:], op=mybir.AluOpType.is_equal)
                        pc_ps = psh.tile([128, 1024], F32, tag="hold")
                        nc.tensor.matmul(out=pc_ps[:, :C2 + 1], lhsT=selB[:],
                                         rhs=h2_sb[:], start=True, stop=True)
                        nc.vector.tensor_tensor(out=poolacc[:], in0=poolacc[:],
                                                in1=pc_ps[:, :C2 + 1],
                                                op=mybir.AluOpType.add)

                stage_a(0)
                if NT > 1:
                    stage_a(1)
                for t in range(NT):
                    if t + 2 < NT:
                        stage_a(t + 2)
                    stage_b1(t)
                    stage_b2(t)
                    stage_c(t)
                return poolacc

            # ================= layer 1 =================
            gat_sweep(1)
            nc.gpsimd.collective_compute(
                "AllGather", mybir.AluOpType.bypass, replica_groups=RG,
                ins=[t2_locC[:, :]],
                outs=[t2_full[NDEV * R2:N, :]])

            # ================= layer 2 + pooling =================
            batchf_sb = cp.tile([128, NT], BF16, tag="batchf")
            nc.sync.dma_start(out=batchf_sb[:], in_=batchf_t[:, :])
            poolacc = gat_sweep(2)

            # ================= pool reduce + FC =================
            nc.sync.dma_start(out=pc_loc[:, :], in_=poolacc[:])
            nc.gpsimd.collective_compute(
                "AllReduce", mybir.AluOpType.add, replica_groups=RG,
                ins=[pc_loc[:, :]], outs=[pc_red[:, :]])
            pc_sb = sp.tile([128, C2 + 1], F32, tag="pc")
            nc.sync.dma_start(out=pc_sb[:], in_=pc_red[:, :])
            cnt = sp.tile([128, 1], F32, tag="cnt")
            nc.vector.tensor_scalar_max(cnt[:], pc_sb[:, C2:C2 + 1], 1.0)
            nc.vector.reciprocal(out=cnt[:], in_=cnt[:])
            g_sb = sp.tile([128, C2], F32, tag="g")
            nc.vector.tensor_scalar_mul(g_sb[:], pc_sb[:, :C2], cnt[:, :1])

            y_ps = psh.tile([128, 1024], F32, tag="hold")
            for c in range(2):
                tp = psa.tile([128, 257], F32, tag="attn")
                nc.tensor.transpose(out=tp[:, :128],
                                    in_=g_sb[:, c * 128:(c + 1) * 128],
                                    identity=ident[:])
                gT = sp.tile([128, 128], F32, tag="gT")
                nc.vector.tensor_copy(out=gT[:], in_=tp[:, :128])
                nc.tensor.matmul(out=y_ps[:, :2], lhsT=gT[:],
                                 rhs=fcW_sb[:, 2 * c:2 * c + 2],
                                 start=(c == 0), stop=False)
            nc.tensor.matmul(out=y_ps[:, :2], lhsT=ones1f[:], rhs=fcb_sb[:],
                             start=False, stop=True)
            y_sb = sp.tile([128, 2], F32, tag="y")
            nc.vector.tensor_copy(out=y_sb[:], in_=y_ps[:, :2])
            nc.sync.dma_start(out=y_t[:, :], in_=y_sb[:])

    nc.compile()
    return nc


def _setup_ntff_hook():
    """The image's antenv lacks axon_hooks; synthesize it and register the
    ctypes NTFF profiling hook so trace=True works."""
    import types
    import antenv
    if hasattr(antenv, "axon_hooks"):
        return
    mod = types.ModuleType("antenv.axon_hooks")
    state = {"hook": None}
    mod.set_axon_ntff_profile_hook = lambda h: state.__setitem__("hook", h)
    mod.get_axon_ntff_profile_hook = lambda: state["hook"]
    sys.modules["antenv.axon_hooks"] = mod
    antenv.axon_hooks = mod
    try:
        from trn_agent_boot.trn_boot import _ntff_profile_via_ctypes
        hook = _ntff_profile_via_ctypes("/opt/axon/libaxon_pjrt.so")
        mod.set_axon_ntff_profile_hook(hook)
    except Exception as e:
        print("ntff hook setup failed:", e)


_CACHE = {}


def kernel(**inputs):
    x = np.ascontiguousarray(np.asarray(inputs["x"], dtype=np.float32))
    edge_index = np.asarray(inputs["edge_index"])
    batch = np.asarray(inputs["batch"])
    W1 = np.ascontiguousarray(np.asarray(inputs["W1"], dtype=np.float32))
    W2 = np.ascontiguousarray(np.asarray(inputs["W2"], dtype=np.float32))
    a_src1 = np.asarray(inputs["a_src1"], dtype=np.float32)
    a_dst1 = np.asarray(inputs["a_dst1"], dtype=np.float32)
    a_src2 = np.asarray(inputs["a_src2"], dtype=np.float32)
    a_dst2 = np.asarray(inputs["a_dst2"], dtype=np.float32)
    b1 = np.asarray(inputs["b1"], dtype=np.float32)
    b2 = np.asarray(inputs["b2"], dtype=np.float32)
    fcW = np.ascontiguousarray(np.asarray(inputs["fcW"], dtype=np.float32))
    fcb = np.asarray(inputs["fcb"], dtype=np.float32)

    (Ks, offs, SK, xidx32, xidx32b, dstf, dstfR, batchf,
     perm_d, perm_l) = _host_prep(edge_index, batch)

    key = (tuple(Ks),)
    if key not in _CACHE:
        _CACHE[key] = _build(Ks, offs, SK)
    nc = _CACHE[key]

    # host-side wa2: [C2IN, 8] with cols [src heads | dst heads]
    W2r = W2.reshape(C2IN, H, C2)
    wa2 = np.concatenate([np.einsum("chd,hd->ch", W2r, a_src2),
                          np.einsum("chd,hd->ch", W2r, a_dst2)], axis=1)
    # host-side asad1 = [x @ (W1 @ a_src1), x @ (W1 @ a_dst1)]  (N x 8)
    W1r = W1.reshape(C1IN, H, C1)
    wa_s1 = np.einsum("chd,hd->ch", W1r, a_src1)
    wa_d1 = np.einsum("chd,hd->ch", W1r, a_dst1)
    asad1 = np.concatenate([x @ wa_s1, x @ wa_d1], axis=1).astype(NPBF)
    xa = np.zeros((N, XROW), dtype=NPBF)
    xa[:, :C1IN] = x.astype(NPBF)
    xa[:, C1IN:C1IN + 8] = asad1
    asadloc = np.zeros((NDEV, NPD, 8), dtype=NPBF)
    asadloc[perm_d, perm_l] = asad1

    in_maps = []
    for d in range(NDEV):
        in_maps.append({
            "xa": xa,
            "W1": W1.astype(NPBF), "W2": W2.astype(NPBF),
            "wa2": wa2.astype(NPBF),
            "fcW": fcW,
            "b1": b1.astype(NPBF), "b2": b2.astype(NPBF),
            "fcb": fcb,
            "xidx32": xidx32[d], "xidx32b": xidx32b[d], "dstf": dstf[d],
            "dstfR": dstfR[d], "batchf": batchf[d],
            "asadloc": asadloc[d],
        })

    import os as _os
    trace = bool(int(_os.environ.get("BASS_GAT_TRACE", "0")))
    kwargs = {}
    if trace:
        _setup_ntff_hook()
        kwargs = dict(trace=True, trace_cores=[0])
    res = run_bass_kernel_spmd(nc, in_maps, core_ids=list(range(NDEV)), **kwargs)
    if trace:
        kernel.last_exec_ns = res.exec_time_ns
        kernel.last_trace = res.instructions_and_trace
        if res.exec_time_ns is not None:
            print(f"HW exec time: {res.exec_time_ns} ns")
    return res.results[0]["y"]


# revision 37
# speedup vs baseline: 1.1516x; 1.1516x over previous
import sys
import numpy as np

sys.path.insert(0, "/opt/trn_rl_repo")
sys.path.insert(0, "/opt/trn_rl_repo/concourse")

import ml_dtypes
import concourse.bass as bass
import concourse.bacc as bacc
import concourse.mybir as mybir
import concourse.tile as tile
from concourse.bass import IndirectOffsetOnAxis
from concourse.bass_utils import run_bass_kernel_spmd
from concourse.masks import make_identity

F32 = mybir.dt.float32
BF16 = mybir.dt.bfloat16
I32 = mybir.dt.int32
I16 = mybir.dt.int16
NPBF = ml_dtypes.bfloat16

N = 20000
E = 160000
B = 128
NDEV = 8
NPD = N // NDEV          # 2500 nodes per device
NT = (NPD + 127) // 128  # 20 dst tiles per device
H = 4
C1IN, C1 = 768, 512
C2IN, C2 = 512, 256
XROW = 784              # [x(768) | as1(4) ad1(4) | pad8] bf16 -> 1568B rows
T2W = 528                # [h1(512) | as2(4) ad2(4) | pad8] bf16 -> 1056B rows
NEG = 0.2
TS1, TS2 = 8, 16         # L1 tile boundaries for the 3-way t2 AllGather
R1 = TS1 * 128           # 896 local rows in part A
R2 = TS2 * 128           # parts: A=[0,896) B=[896,1792) C=[1792,2500)


def _host_prep(edge_index, batch):
    """Integer-only preprocessing: balanced node partition, edge chunk layout."""
    import heapq
    src = np.concatenate([edge_index[0], np.arange(N, dtype=np.int64)]).astype(np.int64)
    dst = np.concatenate([edge_index[1], np.arange(N, dtype=np.int64)]).astype(np.int64)

    # -- degree-balanced assignment of nodes to the 8*NT (device, tile) slots --
    deg = np.bincount(dst, minlength=N)
    tile_rows = [128] * (NT - 1) + [NPD - 128 * (NT - 1)]
    rem = np.array([[r for r in tile_rows] for _ in range(NDEV)], dtype=np.int64)
    fill = np.zeros((NDEV, NT), dtype=np.int64)
    perm_d = np.empty(N, np.int64)
    perm_t = np.empty(N, np.int64)
    perm_s = np.empty(N, np.int64)
    heap = [(0, d * NT + t) for d in range(NDEV) for t in range(NT)]
    heapq.heapify(heap)
    for n in np.argsort(-deg, kind="stable"):
        while True:
            s, dt = heapq.heappop(heap)
            d, t = divmod(dt, NT)
            if rem[d, t] > 0:
                break
        perm_d[n], perm_t[n], perm_s[n] = d, t, fill[d, t]
        fill[d, t] += 1
        rem[d, t] -= 1
        heapq.heappush(heap, (s + int(deg[n]), dt))
    perm_l = perm_t * 128 + perm_s

    ed, et, es = perm_d[dst], perm_t[dst], perm_s[dst]
    cnt = np.zeros((NDEV, NT), dtype=np.int64)
    np.add.at(cnt, (ed, et), 1)
    Ks = [max(1, int(np.ceil(cnt[:, t].max() / 128.0))) for t in range(NT)]
    SK = sum(Ks)
    offs = np.cumsum([0] + Ks)

    # L2 gather table: [devs' rows < R1 | devs' rows in [R1,R2) | devs' rest]
    sd, sl = perm_d[src], perm_l[src]
    src2 = np.where(
        sl < R1, sd * R1 + sl,
        np.where(sl < R2, NDEV * R1 + sd * (R2 - R1) + (sl - R1),
                 NDEV * R2 + sd * (NPD - R2) + (sl - R2)))

    # flat src index per (device, tile): edge j -> lane j%128, chunk j//128
    xidx32 = np.zeros((NDEV, 128, SK), dtype=np.int32)   # L1
    xidx32b = np.zeros((NDEV, 128, SK), dtype=np.int32)  # L2
    dstf = np.full((NDEV, 128, SK), -1.0, dtype=NPBF)
    dstfR = np.full((NDEV, SK, 128), -1.0, dtype=NPBF)   # chunk-major rows

    for d in range(NDEV):
        m = ed == d
        s_d, s2_d, t_d, dl_d = src[m], src2[m], et[m], es[m]
        for t in range(NT):
            mt = t_d == t
            s_t, s2_t = s_d[mt], s2_d[mt]
            dl_t = dl_d[mt]
            o = offs[t]
            j = np.arange(len(s_t))
            xidx32[d, j % 128, o + j // 128] = s_t
            xidx32b[d, j % 128, o + j // 128] = s2_t
            dstf[d, j % 128, o + j // 128] = dl_t.astype(NPBF)
            dstfR[d, o + j // 128, j % 128] = dl_t.astype(NPBF)

    batchf = np.full((NDEV, 128, NT), -1.0, dtype=NPBF)
    b_np = np.asarray(batch).astype(np.int64)
    batchf[perm_d, perm_s, perm_t] = b_np.astype(NPBF)

    return Ks, offs, SK, xidx32, xidx32b, dstf, dstfR, batchf, perm_d, perm_l


def _build_A(a_src, a_dst, cph):
    A = np.zeros((H * cph, 8), dtype=np.float32)
    for h in range(H):
        A[h * cph:(h + 1) * cph, h] = a_src[h]
        A[h * cph:(h + 1) * cph, 4 + h] = a_dst[h]
    return A


def _build(Ks, offs, SK):
    """Emit the Bass program (identical for all 8 cores)."""
    nc = bacc.Bacc("TRN2", target_bir_lowering=False, debug=False, num_devices=NDEV)

    # ---- I/O ----
    xa_t = nc.dram_tensor("xa", [N, XROW], BF16, kind="ExternalInput")
    W1_t = nc.dram_tensor("W1", [C1IN, H * C1], BF16, kind="ExternalInput")
    W2_t = nc.dram_tensor("W2", [C2IN, H * C2], BF16, kind="ExternalInput")
    wa2_t = nc.dram_tensor("wa2", [C2IN, 8], BF16, kind="ExternalInput")
    fcW_t = nc.dram_tensor("fcW", [C2, 2], F32, kind="ExternalInput")
    b1_t = nc.dram_tensor("b1", [C1], BF16, kind="ExternalInput")
    b2_t = nc.dram_tensor("b2", [C2], BF16, kind="ExternalInput")
    fcb_t = nc.dram_tensor("fcb", [2], F32, kind="ExternalInput")
    xidx32_t = nc.dram_tensor("xidx32", [128, SK], I32, kind="ExternalInput")
    xidx32b_t = nc.dram_tensor("xidx32b", [128, SK], I32, kind="ExternalInput")
    dstf_t = nc.dram_tensor("dstf", [128, SK], BF16, kind="ExternalInput")
    dstfR_t = nc.dram_tensor("dstfR", [SK, 128], BF16, kind="ExternalInput")
    batchf_t = nc.dram_tensor("batchf", [128, NT], BF16, kind="ExternalInput")
    asadloc_t = nc.dram_tensor("asadloc", [NPD, 8], BF16, kind="ExternalInput")
    y_t = nc.dram_tensor("y", [B, 2], F32, kind="ExternalOutput")

    # ---- internal DRAM ----
    t2_locA = nc.dram_tensor("t2_locA", [R1, T2W], BF16)
    t2_locB = nc.dram_tensor("t2_locB", [R2 - R1, T2W], BF16)
    t2_locC = nc.dram_tensor("t2_locC", [NPD - R2, T2W], BF16)
    t2_full = nc.dram_tensor("t2_full", [N, T2W], BF16, addr_space="Shared")
    pc_loc = nc.dram_tensor("pc_loc", [B, C2 + 1], F32)
    pc_red = nc.dram_tensor("pc_red", [B, C2 + 1], F32, addr_space="Shared")

    RG = [list(range(NDEV))]
    KMAX = max(Ks)

    with tile.TileContext(nc) as tc:
        with (
            tc.tile_pool(name="const", bufs=1) as cp,
            tc.tile_pool(name="small", bufs=3) as sp,
            tc.tile_pool(name="selp", bufs=3) as selp,
            tc.tile_pool(name="selTp", bufs=3) as selTp,
            tc.tile_pool(name="dstfTp", bufs=3) as dstfTp,
            tc.tile_pool(name="selwp", bufs=3) as selwp,
            tc.tile_pool(name="xgp", bufs=4) as xgp,
            tc.tile_pool(name="utp", bufs=3) as utp,
            tc.tile_pool(name="psu", bufs=2, space="PSUM") as psu,
            tc.tile_pool(name="psh", bufs=1, space="PSUM") as psh,
            tc.tile_pool(name="psa", bufs=2, space="PSUM") as psa,
        ):
            # ================= constants =================
            ident = cp.tile([128, 128], F32, tag="ident")
            make_identity(nc, ident[:])
            iota_i = cp.tile([128, 128], I32, tag="iota_i")
            nc.gpsimd.iota(iota_i[:], pattern=[[1, 128]], base=0, channel_multiplier=0)
            iotaTb = cp.tile([128, 128], BF16, tag="iotaTb")
            nc.vector.tensor_copy(out=iotaTb[:], in_=iota_i[:])
            iota_ci = cp.tile([128, 1], I32, tag="iota_ci")
            nc.gpsimd.iota(iota_ci[:], pattern=[[1, 1]], base=0, channel_multiplier=1)
            iotaCb = cp.tile([128, 1], BF16, tag="iotaCb")
            nc.vector.tensor_copy(out=iotaCb[:], in_=iota_ci[:])
            ones1 = cp.tile([1, 128], BF16, tag="ones1")
            nc.vector.memset(ones1[:], 1.0)
            ones1f = cp.tile([1, 128], F32, tag="ones1f")
            nc.vector.memset(ones1f[:], 1.0)

            b1_sb = cp.tile([1, C1], BF16, tag="b1")
            nc.sync.dma_start(out=b1_sb[:], in_=b1_t[None, :])
            b2_sb = cp.tile([1, C2], BF16, tag="b2")
            nc.sync.dma_start(out=b2_sb[:], in_=b2_t[None, :])
            fcb_sb = cp.tile([1, 2], F32, tag="fcb")
            nc.sync.dma_start(out=fcb_sb[:], in_=fcb_t[None, :])
            fcW_sb = cp.tile([128, 4], F32, tag="fcW")
            for c in range(2):
                nc.sync.dma_start(out=fcW_sb[:, 2 * c:2 * c + 2],
                                  in_=fcW_t[c * 128:(c + 1) * 128, :])
            wa2blk = cp.tile([128, 4 * 8], BF16, tag="wa2blk")
            nc.sync.dma_start(
                out=wa2blk[:].rearrange("p (c j) -> p c j", j=8),
                in_=wa2_t[:].rearrange("(c p) j -> p c j", p=128))

            CC1 = C1IN // 128  # 6
            CC2 = C2IN // 128  # 4
            W1_sb = cp.tile([128, CC1 * H * C1], BF16, tag="W1")
            for c in range(CC1):
                nc.sync.dma_start(out=W1_sb[:, c * H * C1:(c + 1) * H * C1],
                                  in_=W1_t[c * 128:(c + 1) * 128, :])
            W2_sb = cp.tile([128, CC2 * H * C2], BF16, tag="W2")
            for c in range(CC2):
                nc.sync.dma_start(out=W2_sb[:, c * H * C2:(c + 1) * H * C2],
                                  in_=W2_t[c * 128:(c + 1) * 128, :])

            # ================= helper: one GAT layer sweep =================
            # Software-pipelined: stage_a (loads/gather/sel) runs 2 tiles
            # ahead, stage_b1/b2 (attention) 1 tile ahead of stage_c (agg).
            def gat_sweep(layer):
                if layer == 1:
                    CIN, CC, COUT = C1IN, CC1, C1
                    ROW = XROW
                    W_sb, b_sb = W1_sb, b1_sb
                    x_tab, idx_t = xa_t, xidx32_t
                else:
                    CIN, CC, COUT = C2IN, CC2, C2
                    ROW = T2W
                    W_sb, b_sb = W2_sb, b2_sb
                    x_tab, idx_t = t2_full, xidx32b_t

                poolacc = None
                if layer == 2:
                    poolacc = cp.tile([128, C2 + 1], F32, tag="poolacc")
                    nc.vector.memset(poolacc[:], 0.0)

                st = {}

                def t2part(t):
                    if t < TS1:
                        return t2_locA, t * 128
                    if t < TS2:
                        return t2_locB, t * 128 - R1
                    return t2_locC, t * 128 - R2

                def stage_a(t):
                    K, o = Ks[t], offs[t]
                    rows = min(128, NPD - t * 128)
                    xg = xgp.tile([128, KMAX * XROW], BF16, tag="xg")
                    idx_sb = sp.tile([128, KMAX], I32, tag="idx32")
                    nc.sync.dma_start(out=idx_sb[:, :K],
                                      in_=idx_t[:, o:o + K])
                    for k in range(K):
                        nc.gpsimd.indirect_dma_start(
                            out=xg[:, k * ROW:(k + 1) * ROW], out_offset=None,
                            in_=x_tab[:, :],
                            in_offset=IndirectOffsetOnAxis(
                                ap=idx_sb[:, k:k + 1], axis=0))
                    dstf_sb = sp.tile([128, KMAX], BF16, tag="dstf")
                    nc.sync.dma_start(out=dstf_sb[:, :K], in_=dstf_t[:, o:o + K])
                    ad_sb = sp.tile([128, 4], BF16, tag="ad")
                    if rows < 128:
                        nc.vector.memset(ad_sb[:], 0.0)
                    if layer == 1:
                        nc.sync.dma_start(out=ad_sb[:rows, :],
                                          in_=asadloc_t[t * 128:t * 128 + rows, 4:8])
                    else:
                        t2d, r0 = t2part(t)
                        nc.sync.dma_start(out=ad_sb[:rows, :],
                                          in_=t2d[r0:r0 + rows,
                                                  C2IN + 4:C2IN + 8])
                    dstfT = dstfTp.tile([128, KMAX * 128], BF16, tag="dstfT")
                    nc.sync.dma_start(
                        out=dstfT[:, :K * 128].rearrange("p (k d) -> p k d", d=128),
                        in_=dstfR_t[None, o:o + K, :].to_broadcast([128, K, 128]))
                    sel = selp.tile([128, KMAX * 128], BF16, tag="sel")
                    nc.vector.tensor_tensor(
                        out=sel[:, :K * 128].rearrange("p (k d) -> p k d", d=128),
                        in0=dstf_sb[:, :K][:, :, None].to_broadcast([128, K, 128]),
                        in1=iotaTb[:, None, :].to_broadcast([128, K, 128]),
                        op=mybir.AluOpType.is_equal)
                    selT = selTp.tile([128, KMAX * 128], BF16, tag="selT")
                    nc.vector.tensor_tensor(
                        out=selT[:, :K * 128],
                        in0=iotaCb[:, :1].to_broadcast([128, K * 128]),
                        in1=dstfT[:, :K * 128], op=mybir.AluOpType.is_equal)
                    st[t] = dict(xg=xg, sel=sel, selT=selT, ad=ad_sb)

                def stage_b1(t):
                    K = Ks[t]
                    s = st[t]
                    xg, sel, selT, ad_sb = s["xg"], s["sel"], s["selT"], s["ad"]
                    ade_ps = psa.tile([128, 257], F32, tag="attn")
                    for k in range(K):
                        nc.tensor.matmul(out=ade_ps[:, 4 * k:4 * k + 4],
                                         lhsT=selT[:, 128 * k:128 * (k + 1)],
                                         rhs=ad_sb[:],
                                         start=(k == 0), stop=(k == K - 1))
                    xg3 = xg[:, :K * ROW].rearrange("p (k d) -> p k d", d=ROW)
                    z = sp.tile([128, 4 * KMAX], F32, tag="z")
                    z3 = z[:, :4 * K].rearrange("p (k s) -> p k s", s=4)
                    nc.vector.tensor_tensor(out=z3, in0=xg3[:, :, CIN:CIN + 4],
                                            in1=ade_ps[:, :4 * K].rearrange(
                                                "p (k s) -> p k s", s=4),
                                            op=mybir.AluOpType.add)
                    zs = sp.tile([128, 4 * KMAX], F32, tag="zs")
                    nc.vector.tensor_scalar_mul(zs[:, :4 * K], z[:, :4 * K], NEG)
                    nc.vector.tensor_tensor(out=z[:, :4 * K], in0=z[:, :4 * K],
                                            in1=zs[:, :4 * K], op=mybir.AluOpType.max)
                    ex = sp.tile([128, 4 * KMAX], BF16, tag="ex")
                    nc.scalar.activation(out=ex[:, :4 * K], in_=z[:, :4 * K],
                                         func=mybir.ActivationFunctionType.Exp)
                    s["ex"] = ex

                def stage_b2(t):
                    K = Ks[t]
                    s = st[t]
                    sel, selT, ex = s["sel"], s["selT"], s["ex"]
                    den_ps = psa.tile([128, 257], F32, tag="attn")
                    for k in range(K):
                        nc.tensor.matmul(out=den_ps[:, 0:4],
                                         lhsT=sel[:, 128 * k:128 * (k + 1)],
                                         rhs=ex[:, 4 * k:4 * k + 4],
                                         start=(k == 0), stop=(k == K - 1))
                    den_sb = sp.tile([128, 4], BF16, tag="den")
                    nc.scalar.activation(out=den_sb[:], in_=den_ps[:, 0:4],
                                         func=mybir.ActivationFunctionType.Copy)
                    dene_ps = psa.tile([128, 257], F32, tag="attn")
                    for k in range(K):
                        nc.tensor.matmul(out=dene_ps[:, 4 * k:4 * k + 4],
                                         lhsT=selT[:, 128 * k:128 * (k + 1)],
                                         rhs=den_sb[:],
                                         start=(k == 0), stop=(k == K - 1))
                    rden = sp.tile([128, 4 * KMAX], F32, tag="rden")
                    nc.vector.tensor_scalar(out=rden[:, :4 * K],
                                            in0=dene_ps[:, :4 * K],
                                            scalar1=4.0, scalar2=1e-20,
                                            op0=mybir.AluOpType.mult,
                                            op1=mybir.AluOpType.max)
                    nc.vector.reciprocal(out=rden[:, :4 * K], in_=rden[:, :4 * K])
                    alpha = sp.tile([128, 4 * KMAX], BF16, tag="alpha")
                    nc.vector.tensor_tensor(out=alpha[:, :4 * K], in0=ex[:, :4 * K],
                                            in1=rden[:, :4 * K],
                                            op=mybir.AluOpType.mult)
                    selw = selwp.tile([128, KMAX * 512], BF16, tag="selw")
                    nc.vector.tensor_tensor(
                        out=selw[:, :K * 512].rearrange(
                            "p (k h d) -> p k h d", h=4, d=128),
                        in0=sel[:, :K * 128].rearrange(
                            "p (k d) -> p k d", d=128)[:, :, None, :].to_broadcast(
                            [128, K, 4, 128]),
                        in1=alpha[:, :4 * K].rearrange(
                            "p (k h) -> p k h", h=4)[:, :, :, None].to_broadcast(
                            [128, K, 4, 128]),
                        op=mybir.AluOpType.mult)
                    s["selw"] = selw

                def stage_c(t):
                    K = Ks[t]
                    rows = min(128, NPD - t * 128)
                    s = st.pop(t)
                    xg, selw = s["xg"], s["selw"]
                    out_ps = psh.tile([128, 1024], F32, tag="hold")
                    firsts = [True, True]
                    NPASS = CC // 2
                    for p_i in range(NPASS):
                        ut_ps = psu.tile([128, 1024], F32, tag="ut")
                        for k in range(K):
                            for ci in range(2):
                                c = 2 * p_i + ci
                                nc.tensor.matmul(
                                    out=ut_ps[:, ci * 512:(ci + 1) * 512],
                                    lhsT=xg[:, k * ROW + c * 128:
                                            k * ROW + (c + 1) * 128],
                                    rhs=selw[:, k * 512:(k + 1) * 512],
                                    start=(k == 0), stop=(k == K - 1))
                        ut_sb = utp.tile([128, 1024], BF16, tag="ut")
                        nc.scalar.activation(out=ut_sb[:],
                                             in_=ut_ps[:],
                                             func=mybir.ActivationFunctionType.Copy)
                        for ci in range(2):
                            c = 2 * p_i + ci
                            for h in range(H):
                                half = h % 2
                                last1 = (half == 1 and p_i == NPASS - 1
                                         and ci == 1 and h == 3)
                                nc.tensor.matmul(
                                    out=out_ps[:, half * 512:half * 512 + COUT],
                                    lhsT=ut_sb[:, ci * 512 + h * 128:
                                               ci * 512 + (h + 1) * 128],
                                    rhs=W_sb[:, c * H * COUT + h * COUT:
                                             c * H * COUT + (h + 1) * COUT],
                                    start=firsts[half], stop=last1)
                                firsts[half] = False
                    nc.tensor.matmul(out=out_ps[:, 0:COUT], lhsT=ones1[:],
                                     rhs=b_sb[:], start=False, stop=True)

                    oh1 = sp.tile([128, COUT], F32, tag="oh1")
                    nc.scalar.activation(out=oh1[:], in_=out_ps[:, 512:512 + COUT],
                                         func=mybir.ActivationFunctionType.Copy)
                    if layer == 1:
                        h1f_sb = sp.tile([128, C1], F32, tag="h1f")
                        nc.vector.tensor_tensor(out=h1f_sb[:],
                                                in0=out_ps[:, 0:COUT],
                                                in1=oh1[:],
                                                op=mybir.AluOpType.add)
                        h1_sb = sp.tile([128, C1], BF16, tag="h1")
                        nc.scalar.activation(out=h1_sb[:], in_=h1f_sb[:],
                                             func=mybir.ActivationFunctionType.Copy)
                        # asad2 = h1 @ wa2 via PE transposes of h1
                        as2_ps = psh.tile([128, 1024], F32, tag="hold")
                        for c in range(CC2):
                            tp = psa.tile([128, 257], F32, tag="attn")
                            nc.tensor.transpose(out=tp[:, :128],
                                                in_=h1f_sb[:, c * 128:(c + 1) * 128],
                                                identity=ident[:])
                            h1T = sp.tile([128, 128], BF16, tag="h1T")
                            nc.vector.tensor_copy(out=h1T[:], in_=tp[:, :128])
                            nc.tensor.matmul(out=as2_ps[:, 0:8], lhsT=h1T[:],
                                             rhs=wa2blk[:, c * 8:(c + 1) * 8],
                                             start=(c == 0), stop=(c == CC2 - 1))
                        as2_sb = sp.tile([128, 8], BF16, tag="as2")
                        nc.vector.tensor_copy(out=as2_sb[:], in_=as2_ps[:, 0:8])
                        t2d, r0 = t2part(t)
                        nc.sync.dma_start(out=t2d[r0:r0 + rows, 0:C2IN],
                                          in_=h1_sb[:rows, :])
                        nc.sync.dma_start(out=t2d[r0:r0 + rows, C2IN:C2IN + 8],
                                          in_=as2_sb[:rows, :])
                        if t == TS1 - 1:
                            nc.gpsimd.collective_compute(
                                "AllGather", mybir.AluOpType.bypass,
                                replica_groups=RG,
                                ins=[t2_locA[:, :]],
                                outs=[t2_full[0:NDEV * R1, :]])
                        elif t == TS2 - 1:
                            nc.gpsimd.collective_compute(
                                "AllGather", mybir.AluOpType.bypass,
                                replica_groups=RG,
                                ins=[t2_locB[:, :]],
                                outs=[t2_full[NDEV * R1:NDEV * R2, :]])
                    else:
                        h2_sb = sp.tile([128, C2 + 1], BF16, tag="h2")
                        nc.vector.tensor_tensor(out=h2_sb[:, :C2],
                                                in0=out_ps[:, 0:COUT],
                                                in1=oh1[:, :COUT],
                                                op=mybir.AluOpType.add)
                        nc.vector.memset(h2_sb[:, C2:C2 + 1], 1.0)
                        selB = sp.tile([128, 128], BF16, tag="selB")
                        nc.vector.tensor_tensor(
                            out=selB[:],
                            in0=batchf_sb[:, t:t + 1].to_broadcast([128, 128]),
                            in1=iotaTb[:], op=mybir.AluOpType.is_equal)
                        pc_ps = psh.tile([128, 1024], F32, tag="hold")
                        nc.tensor.matmul(out=pc_ps[:, :C2 + 1], lhsT=selB[:],
                                         rhs=h2_sb[:], start=True, stop=True)
                        nc.vector.tensor_tensor(out=poolacc[:], in0=poolacc[:],
                                                in1=pc_ps[:, :C2 + 1],
                                                op=mybir.AluOpType.add)

                stage_a(0)
                if NT > 1:
                    stage_a(1)
                for t in range(NT):
                    if t + 2 < NT:
                        stage_a(t + 2)
                    stage_b1(t)
                    stage_b2(t)
                    stage_c(t)
                return poolacc

            # ================= layer 1 =================
            gat_sweep(1)
            nc.gpsimd.collective_compute(
                "AllGather", mybir.AluOpType.bypass, replica_groups=RG,
                ins=[t2_locC[:, :]],
                outs=[t2_full[NDEV * R2:N, :]])

            # ================= layer 2 + pooling =================
            batchf_sb = cp.tile([128, NT], BF16, tag="batchf")
            nc.sync.dma_start(out=batchf_sb[:], in_=batchf_t[:, :])
            poolacc = gat_sweep(2)

            # ================= pool reduce + FC =================
            nc.sync.dma_start(out=pc_loc[:, :], in_=poolacc[:])
            nc.gpsimd.collective_compute(
                "AllReduce", mybir.AluOpType.add, replica_groups=RG,
                ins=[pc_loc[:, :]], outs=[pc_red[:, :]])
            pc_sb = sp.tile([128, C2 + 1], F32, tag="pc")
            nc.sync.dma_start(out=pc_sb[:], in_=pc_red[:, :])
            cnt = sp.tile([128, 1], F32, tag="cnt")
            nc.vector.tensor_scalar_max(cnt[:], pc_sb[:, C2:C2 + 1], 1.0)
            nc.vector.reciprocal(out=cnt[:], in_=cnt[:])
            g_sb = sp.tile([128, C2], F32, tag="g")
            nc.vector.tensor_scalar_mul(g_sb[:], pc_sb[:, :C2], cnt[:, :1])

            y_ps = psh.tile([128, 1024], F32, tag="hold")
            for c in range(2):
                tp = psa.tile([128, 257], F32, tag="attn")
                nc.tensor.transpose(out=tp[:, :128],
                                    in_=g_sb[:, c * 128:(c + 1) * 128],
                                    identity=ident[:])
                gT = sp.tile([128, 128], F32, tag="gT")
                nc.vector.tensor_copy(out=gT[:], in_=tp[:, :128])
                nc.tensor.matmul(out=y_ps[:, :2], lhsT=gT[:],
                                 rhs=fcW_sb[:, 2 * c:2 * c + 2],
                                 start=(c == 0), stop=False)
            nc.tensor.matmul(out=y_ps[:, :2], lhsT=ones1f[:], rhs=fcb_sb[:],
                             start=False, stop=True)
            y_sb = sp.tile([128, 2], F32, tag="y")
            nc.vector.tensor_copy(out=y_sb[:], in_=y_ps[:, :2])
            nc.sync.dma_start(out=y_t[:, :], in_=y_sb[:])

    nc.compile()
    return nc


def _setup_ntff_hook():
    """The image's antenv lacks axon_hooks; synthesize it and register the
    ctypes NTFF profiling hook so trace=True works."""
    import types
    import antenv
    if hasattr(antenv, "axon_hooks"):
        return
    mod = types.ModuleType("antenv.axon_hooks")
    state = {"hook": None}
    mod.set_axon_ntff_profile_hook = lambda h: state.__setitem__("hook", h)
    mod.get_axon_ntff_profile_hook = lambda: state["hook"]
    sys.modules["antenv.axon_hooks"] = mod
    antenv.axon_hooks = mod
    try:
        from trn_agent_boot.trn_boot import _ntff_profile_via_ctypes
        hook = _ntff_profile_via_ctypes("/opt/axon/libaxon_pjrt.so")
        mod.set_axon_ntff_profile_hook(hook)
    except Exception as e:
        print("ntff hook setup failed:", e)


_CACHE = {}


def kernel(**inputs):
    x = np.ascontiguousarray(np.asarray(inputs["x"], dtype=np.float32))
    edge_index = np.asarray(inputs["edge_index"])
    batch = np.asarray(inputs["batch"])
    W1 = np.ascontiguousarray(np.asarray(inputs["W1"], dtype=np.float32))
    W2 = np.ascontiguousarray(np.asarray(inputs["W2"], dtype=np.float32))
    a_src1 = np.asarray(inputs["a_src1"], dtype=np.float32)
    a_dst1 = np.asarray(inputs["a_dst1"], dtype=np.float32)
    a_src2 = np.asarray(inputs["a_src2"], dtype=np.float32)
    a_dst2 = np.asarray(inputs["a_dst2"], dtype=np.float32)
    b1 = np.asarray(inputs["b1"], dtype=np.float32)
    b2 = np.asarray(inputs["b2"], dtype=np.float32)
    fcW = np.ascontiguousarray(np.asarray(inputs["fcW"], dtype=np.float32))
    fcb = np.asarray(inputs["fcb"], dtype=np.float32)

    (Ks, offs, SK, xidx32, xidx32b, dstf, dstfR, batchf,
     perm_d, perm_l) = _host_prep(edge_index, batch)

    key = (tuple(Ks),)
    if key not in _CACHE:
        _CACHE[key] = _build(Ks, offs, SK)
    nc = _CACHE[key]

    # host-side wa2: [C2IN, 8] with cols [src heads | dst heads]
    W2r = W2.reshape(C2IN, H, C2)
    wa2 = np.concatenate([np.einsum("chd,hd->ch", W2r, a_src2),
                          np.einsum("chd,hd->ch", W2r, a_dst2)], axis=1)
    # host-side asad1 = [x @ (W1 @ a_src1), x @ (W1 @ a_dst1)]  (N x 8)
    W1r = W1.reshape(C1IN, H, C1)
    wa_s1 = np.einsum("chd,hd->ch", W1r, a_src1)
    wa_d1 = np.einsum("chd,hd->ch", W1r, a_dst1)
    asad1 = np.concatenate([x @ wa_s1, x @ wa_d1], axis=1).astype(NPBF)
    xa = np.zeros((N, XROW), dtype=NPBF)
    xa[:, :C1IN] = x.astype(NPBF)
    xa[:, C1IN:C1IN + 8] = asad1
    asadloc = np.zeros((NDEV, NPD, 8), dtype=NPBF)
    asadloc[perm_d, perm_l] = asad1

    in_maps = []
    for d in range(NDEV):
        in_maps.append({
            "xa": xa,
            "W1": W1.astype(NPBF), "W2": W2.astype(NPBF),
            "wa2": wa2.astype(NPBF),
            "fcW": fcW,
            "b1": b1.astype(NPBF), "b2": b2.astype(NPBF),
            "fcb": fcb,
            "xidx32": xidx32[d], "xidx32b": xidx32b[d], "dstf": dstf[d],
            "dstfR": dstfR[d], "batchf": batchf[d],
            "asadloc": asadloc[d],
        })

    import os as _os
    trace = bool(int(_os.environ.get("BASS_GAT_TRACE", "0")))
    kwargs = {}
    if trace:
        _setup_ntff_hook()
        kwargs = dict(trace=True, trace_cores=[0])
    res = run_bass_kernel_spmd(nc, in_maps, core_ids=list(range(NDEV)), **kwargs)
    if trace:
        kernel.last_exec_ns = res.exec_time_ns
        kernel.last_trace = res.instructions_and_trace
        if res.exec_time_ns is not None:
            print(f"HW exec time: {res.exec_time_ns} ns")
    return res.results[0]["y"]
